# revision 5
# baseline (speedup 1.0000x reference)
"""Trainium2 Bass kernel for nn_DKWinners (per-segment argmax one-hot mask * x).

Dual-engine split of the 2-pass compact-key design:

The segmented argmax is computed as float-max over per-element KEYS that
embed (value, lane). Two key encodings, one per engine, split by column
tile to balance engine busy time:

DVE tiles (bit keys):   key = (bits(x) & ~15) | (15 - lane)
  via custom DVE op  out = x ^ ((x ^ code) & 0xF)  (bitwise ALU ops, lane
  code tile holds raw denormal patterns 0..15).

Scalar tiles (integer keys): key = 16*q + (15 - lane),  q = round(x * 2^17)
  via 3 Copy-activations on the Activation engine:
    A: a = x + 96     (rounds x to 2^-17 grid: [88,104) has ulp 2^-17)
    B: b = a - 96     (exact, Sterbenz)
    C: k_l = b_l * 2^21 + (15-l)   per lane slice (strided)
  All integer-exact in f32 (|16q| <= 2^24 for |x| < 8).

Both key kinds: float-max over a segment picks the largest truncated/
quantized value; lane code breaks near-ties. One native DVE tensor_reduce
(max) per tile gives the compact winner key [128, S].

Output: kout [128, 4096] f32 per core. Host decodes lane+value per column
range and scatters into the zero output (pure unshard/layout step).

Engine busy per core (theory): DVE = pack(10 tiles) + reduce(16) ~ 115us,
Scalar = 6 tiles * ~17.6us ~ 106us, DMA-in ~ 94us -> wall ~ 120us.
"""

from contextlib import ExitStack

import numpy as np

ROWS = 1024
N = 65536
DPC = 16
OUT_DIM = N // DPC  # 4096
N_CORES = 8
ROWS_PER_CORE = ROWS // N_CORES  # 128 -> partition dim

F = 4096          # free-dim tile size (per partition)
GRID = 17         # scalar-key quantization: 2^-17 grid
RND = 96.0        # rounding bias: [88,104) ulp = 2^-17
SCALE = float(1 << (GRID + 4))  # 2^21: b*2^21 = 16*q_int

_cache = {}
_dve_ops = {}

# tile schedule (shared by device build + host decode):
# taper halves at the ends; scalar engine handles SCALAR_TILES (full tiles)
_HALF = F // 2
_SIZES = [_HALF, _HALF] + [F] * ((N - 2 * F) // F) + [_HALF, _HALF]
SCALAR_TILES = frozenset({3, 5, 7, 9, 11, 13})


def _register_dve_ops():
    """Define + register the bitwise KEYPACK custom DVE op (idempotent)."""
    if _dve_ops:
        return _dve_ops

    from concourse import dve_ops
    from concourse.dve_spec import Bin, C0, C1, Spec, Src0, Src1, AluOp, lower
    from concourse.dve_table_gen import free_opcode_rows
    from concourse.dve_uop import DveOpSpec

    def _ref_keypack(in0, in1, s0, s1, imm2):
        x = np.asarray(in0, np.float32)
        c = np.asarray(in1, np.float32)
        m = np.float32(s0).view(np.uint32) ^ np.float32(s1).view(np.uint32)
        r = x.view(np.uint32) ^ ((x.view(np.uint32) ^ c.view(np.uint32)) & m)
        return r.view(np.float32)

    AND = AluOp.BITWISE_AND
    XOR = AluOp.BITWISE_XOR
    specs = {
        # out = x ^ ((x ^ code) & 0xF)  ==  (bits(x) & ~15) | code
        # mask 0xF delivered as XOR of two *normal* f32 constants (the
        # scalar-constant path canonicalizes NaN patterns; normals are safe).
        "SEG_KEYPACK_ANT": Spec(
            body=Bin(
                XOR,
                Src0,
                Bin(AND, Bin(XOR, Src0, Src1), Bin(XOR, C0, C1)),
            ),
            reference=_ref_keypack,
        ),
    }

    next_row = max(dve_ops._SUB_OPCODE_FOR_NAME.values()) + 1
    free_rows = set(free_opcode_rows("TRN2"))
    for name, spec in specs.items():
        if name in dve_ops._SUB_OPCODE_FOR_NAME:
            _dve_ops[name] = next(o for o in dve_ops.OPS if o.name == name)
            continue
        row = next_row
        next_row += 1
        assert row in free_rows, (row, sorted(free_rows))
        shas = {}
        for ver in ("v3", "v4"):
            try:
                uops = lower(spec, ver=ver)
            except Exception:
                continue
            shas[ver] = DveOpSpec(
                name=name, opcode=row, uops=uops, rd1_en=True
            ).sha(ver)
        op = dve_ops.DveOp(name, spec, subdim=False, uops_sha=shas)
        dve_ops._SUB_OPCODE_FOR_NAME[name] = row
        dve_ops.OPS.append(op)
        dve_ops.CUSTOM_DVE_SPECS[name] = spec
        _dve_ops[name] = op
    return _dve_ops


def _build_nc(n_cols):
    import concourse.tile as tile
    from concourse import bacc, mybir

    ops = _register_dve_ops()
    kp_op = ops["SEG_KEYPACK_ANT"]

    dt = mybir.dt
    alu = mybir.AluOpType
    Copy = mybir.ActivationFunctionType.Copy

    nc = bacc.Bacc(
        "TRN2",
        target_bir_lowering=False,
        debug=False,
        enable_asserts=False,
    )
    x = nc.dram_tensor("x", [128, n_cols], dt.float32, kind="ExternalInput").ap()
    codes = nc.dram_tensor("codes", [128, DPC], dt.float32, kind="ExternalInput").ap()
    kout = nc.dram_tensor(
        "kout", [128, n_cols // DPC], dt.float32, kind="ExternalOutput"
    ).ap()

    sizes = _SIZES
    assert sum(sizes) == n_cols

    with tile.TileContext(nc) as tc, ExitStack() as ctx:
        cp = ctx.enter_context(tc.tile_pool(name="cp", bufs=1))
        xp = ctx.enter_context(tc.tile_pool(name="xt", bufs=3))
        kp = ctx.enter_context(tc.tile_pool(name="kt", bufs=2))
        sxp = ctx.enter_context(tc.tile_pool(name="sx", bufs=2))
        skp = ctx.enter_context(tc.tile_pool(name="sk", bufs=2))
        rp = ctx.enter_context(tc.tile_pool(name="rt", bufs=4))

        ct = cp.tile([128, DPC], dt.float32, tag="ct")
        nc.sync.dma_start(ct[:], codes)
        c3 = ct[:].rearrange("p (o l) -> p o l", o=1)
        c0_lit = float(np.uint32(0x4000000F).view(np.float32))

        off = 0
        for ti, fi in enumerate(sizes):
            s = fi // DPC
            if ti in SCALAR_TILES:
                # ---- Scalar-engine (Activation) integer-key chain ----
                sxt = sxp.tile([128, fi], dt.float32, tag="sx")
                nc.sync.dma_start(sxt[:], x[:, off : off + fi])
                skt = skp.tile([128, fi], dt.float32, tag="sk")
                # A: sk = x + 96 (quantize to 2^-17 grid)
                nc.scalar.activation(skt[:], sxt[:], Copy, bias=RND, scale=1.0)
                # B: sx = sk - 96 (exact) -- x tile is dead, reuse it
                nc.scalar.activation(sxt[:], skt[:], Copy, bias=-RND, scale=1.0)
                # C: per lane slice: sk[:,:,l] = sx[:,:,l]*2^21 + (15-l)
                b3 = sxt[:].rearrange("p (s l) -> p s l", l=DPC)
                k3s = skt[:].rearrange("p (s l) -> p s l", l=DPC)
                for l in range(DPC):
                    nc.scalar.activation(
                        k3s[:, :, l : l + 1],
                        b3[:, :, l : l + 1],
                        Copy,
                        bias=float(DPC - 1 - l),
                        scale=SCALE,
                    )
                key_tile = skt
            else:
                # ---- DVE bitwise keypack ----
                xt = xp.tile([128, fi], dt.float32, tag="xt")
                nc.sync.dma_start(xt[:], x[:, off : off + fi])
                kt = kp.tile([128, fi], dt.float32, tag="kt")
                nc.vector._custom_dve(
                    kp_op,
                    out=kt[:],
                    in0=xt[:],
                    in1=c3.broadcast_to((128, s, DPC)),
                    s0=c0_lit,
                    s1=2.0,
                )
                key_tile = kt

            rt = rp.tile([128, s], dt.float32, tag="rt")
            r3 = rt[:].rearrange("p (s o) -> p s o", o=1)
            kv = key_tile[:].rearrange("p (s l) -> p s l", l=DPC)
            nc.vector.tensor_reduce(r3, kv, axis=mybir.AxisListType.X, op=alu.max)

            nc.sync.dma_start(kout[:, off // DPC : off // DPC + s], rt[:])
            off += fi

    nc.compile()
    return nc


def _get_nc(n_cols=N):
    if n_cols not in _cache:
        _cache[n_cols] = _build_nc(n_cols)
    return _cache[n_cols]


def _consts():
    codes = np.broadcast_to(
        (15 - np.arange(DPC, dtype=np.uint32))[None, :], (128, DPC)
    ).copy().view(np.float32)
    return codes


def _in_maps(x):
    codes = _consts()
    return [
        {
            "x": x[i * ROWS_PER_CORE : (i + 1) * ROWS_PER_CORE],
            "codes": codes,
        }
        for i in range(N_CORES)
    ]


def _scalar_col_mask():
    """Boolean [OUT_DIM]: which output segments came from scalar tiles."""
    m = np.zeros(OUT_DIM, bool)
    off = 0
    for ti, fi in enumerate(_SIZES):
        s = fi // DPC
        if ti in SCALAR_TILES:
            m[off : off + s] = True
        off += s
    return m


def kernel(x):
    from concourse import bass_utils

    x = np.ascontiguousarray(x, dtype=np.float32)
    assert x.shape == (ROWS, N), x.shape
    nc = _get_nc(N)
    res = bass_utils.run_bass_kernel_spmd(nc, _in_maps(x), core_ids=list(range(N_CORES)))
    K = np.concatenate([r["kout"] for r in res.results], axis=0)  # [ROWS, OUT_DIM] f32

    lane = np.empty((ROWS, OUT_DIM), np.intp)
    val = np.empty((ROWS, OUT_DIM), np.float32)

    sm = _scalar_col_mask()
    # DVE bit-keys
    kb = K[:, ~sm].view(np.uint32)
    lane[:, ~sm] = 15 - (kb & np.uint32(15))
    val[:, ~sm] = (kb & np.uint32(0xFFFFFFF0)).view(np.float32)
    # Scalar integer keys: k = 16*q + (15-lane), q = round(x*2^17)
    ki = K[:, sm].astype(np.int64)
    c = np.mod(ki, 16)
    lane[:, sm] = 15 - c
    val[:, sm] = ((ki - c) // 16).astype(np.float32) * np.float32(2.0 ** -GRID)

    out = np.zeros((ROWS, OUT_DIM, DPC), np.float32)
    np.put_along_axis(out, lane[:, :, None], val[:, :, None], axis=2)
    return out.reshape(ROWS, N)


# revision 6
# speedup vs baseline: 1.0126x; 1.0126x over previous
"""Trainium2 Bass kernel for nn_DKWinners (per-segment argmax one-hot mask * x).

Bit-pack design (2 DVE passes instead of 4, compact output):

Per core (batch-sharded, 128 rows -> partition dim), per column tile [128, F]:
  1. KEYPACK (custom DVE, bitwise): key = (bits(x) & 0xFFFFFFF0) | (15 - lane)
     - low 4 mantissa bits of x replaced by a lane code, descending in lane.
     - float-compare order of keys == order of x truncated to 19 mantissa
       bits; among trunc-equal elements the FIRST lane wins for positive x
       (largest code) and the last lane for negative x (smaller |mantissa|
       is the larger negative). Tie regions are ~2^-19 relative, so the
       winner can differ from exact argmax only when the top-2 gap is
       < ~2e-6 (rel err contribution ~3e-3 worst case, tolerance 2e-2).
  2. native tensor_reduce(max) over [128, S, 16] -> K [128, S] winner keys.

Output: K [128, 4096] f32 per core (2 MiB instead of 32 MiB dense).
Host decode (pure unshard/layout): lane = 15 - (bits & 15);
value = f32(bits & ~15); scatter into zeros at [row, seg, lane].

HBM traffic/core: 32 MiB in + 2 MiB out (vs 64 MiB dense);
DVE: 2 passes (~8.5 us/tile) vs 4.
"""

import numpy as np

ROWS = 1024
N = 65536
DPC = 16
OUT_DIM = N // DPC  # 4096
N_CORES = 8
ROWS_PER_CORE = ROWS // N_CORES  # 128 -> partition dim

F = 4096          # free-dim tile size (per partition)

_cache = {}
_dve_ops = {}


def _register_dve_ops():
    """Define + register the KEYPACK custom DVE op (idempotent)."""
    if _dve_ops:
        return _dve_ops

    from concourse import dve_ops
    from concourse.dve_spec import Bin, C0, C1, Spec, Src0, Src1, AluOp, lower
    from concourse.dve_table_gen import free_opcode_rows
    from concourse.dve_uop import DveOpSpec

    def _ref_keypack(in0, in1, s0, s1, imm2):
        x = np.asarray(in0, np.float32)
        c = np.asarray(in1, np.float32)
        m = np.float32(s0).view(np.uint32) ^ np.float32(s1).view(np.uint32)
        r = x.view(np.uint32) ^ ((x.view(np.uint32) ^ c.view(np.uint32)) & m)
        return r.view(np.float32)

    AND = AluOp.BITWISE_AND
    XOR = AluOp.BITWISE_XOR
    specs = {
        # out = x ^ ((x ^ code) & 0xF)  ==  (bits(x) & ~15) | code
        # mask 0xF delivered as XOR of two *normal* f32 constants (the
        # scalar-constant path canonicalizes NaN patterns; normals are safe).
        "SEG_KEYPACK_ANT": Spec(
            body=Bin(
                XOR,
                Src0,
                Bin(AND, Bin(XOR, Src0, Src1), Bin(XOR, C0, C1)),
            ),
            reference=_ref_keypack,
        ),
    }

    next_row = max(dve_ops._SUB_OPCODE_FOR_NAME.values()) + 1
    free_rows = set(free_opcode_rows("TRN2"))
    for name, spec in specs.items():
        if name in dve_ops._SUB_OPCODE_FOR_NAME:
            _dve_ops[name] = next(o for o in dve_ops.OPS if o.name == name)
            continue
        row = next_row
        next_row += 1
        assert row in free_rows, (row, sorted(free_rows))
        # compute the uops sha for every ver so DveOp.compile's pin check passes
        shas = {}
        for ver in ("v3", "v4"):
            try:
                uops = lower(spec, ver=ver)
            except Exception:
                continue
            shas[ver] = DveOpSpec(
                name=name, opcode=row, uops=uops, rd1_en=True
            ).sha(ver)
        op = dve_ops.DveOp(name, spec, subdim=False, uops_sha=shas)
        dve_ops._SUB_OPCODE_FOR_NAME[name] = row
        dve_ops.OPS.append(op)
        dve_ops.CUSTOM_DVE_SPECS[name] = spec
        _dve_ops[name] = op
    return _dve_ops


def _build_nc(n_cols):
    from contextlib import ExitStack

    import concourse.tile as tile
    from concourse import bacc, mybir

    ops = _register_dve_ops()
    kp_op = ops["SEG_KEYPACK_ANT"]

    dt = mybir.dt
    alu = mybir.AluOpType

    nc = bacc.Bacc(
        "TRN2",
        target_bir_lowering=False,
        debug=False,
        enable_asserts=False,
    )
    x = nc.dram_tensor("x", [128, n_cols], dt.float32, kind="ExternalInput").ap()
    codes = nc.dram_tensor("codes", [128, DPC], dt.float32, kind="ExternalInput").ap()
    kout = nc.dram_tensor(
        "kout", [128, n_cols // DPC], dt.float32, kind="ExternalOutput"
    ).ap()

    # tapered schedule: half-size tiles at both ends shorten pipeline
    # fill (first load) and drain (last store); full F tiles in the middle
    half = F // 2
    assert n_cols % F == 0 and n_cols >= 2 * F
    sizes = [half, half] + [F] * ((n_cols - 2 * F) // F) + [half, half]
    assert sum(sizes) == n_cols

    with tile.TileContext(nc) as tc, ExitStack() as ctx:
        cp = ctx.enter_context(tc.tile_pool(name="cp", bufs=1))
        xp = ctx.enter_context(tc.tile_pool(name="xt", bufs=4))
        kp = ctx.enter_context(tc.tile_pool(name="kt", bufs=3))
        rp = ctx.enter_context(tc.tile_pool(name="rt", bufs=3))

        ct = cp.tile([128, DPC], dt.float32, tag="ct")
        nc.sync.dma_start(ct[:], codes)
        c3 = ct[:].rearrange("p (o l) -> p o l", o=1)
        c0_lit = float(np.uint32(0x4000000F).view(np.float32))

        off = 0
        for fi in sizes:
            s = fi // DPC
            xt = xp.tile([128, fi], dt.float32, tag="xt")
            nc.sync.dma_start(xt[:], x[:, off : off + fi])

            kt = kp.tile([128, fi], dt.float32, tag="kt")
            nc.vector._custom_dve(
                kp_op,
                out=kt[:],
                in0=xt[:],
                in1=c3.broadcast_to((128, s, DPC)),
                s0=c0_lit,
                s1=2.0,
            )

            rt = rp.tile([128, s], dt.float32, tag="rt")
            r3 = rt[:].rearrange("p (s o) -> p s o", o=1)
            kv = kt[:].rearrange("p (s l) -> p s l", l=DPC)
            nc.vector.tensor_reduce(r3, kv, axis=mybir.AxisListType.X, op=alu.max)

            nc.sync.dma_start(kout[:, off // DPC : off // DPC + s], rt[:])
            off += fi

    nc.compile()
    return nc


def _get_nc(n_cols=N):
    if n_cols not in _cache:
        _cache[n_cols] = _build_nc(n_cols)
    return _cache[n_cols]


def _consts():
    codes = np.broadcast_to(
        (15 - np.arange(DPC, dtype=np.uint32))[None, :], (128, DPC)
    ).copy().view(np.float32)
    return codes


def _in_maps(x):
    codes = _consts()
    return [
        {
            "x": x[i * ROWS_PER_CORE : (i + 1) * ROWS_PER_CORE],
            "codes": codes,
        }
        for i in range(N_CORES)
    ]


def kernel(x):
    from concourse import bass_utils

    x = np.ascontiguousarray(x, dtype=np.float32)
    assert x.shape == (ROWS, N), x.shape
    nc = _get_nc(N)
    res = bass_utils.run_bass_kernel_spmd(nc, _in_maps(x), core_ids=list(range(N_CORES)))
    kbits = np.concatenate([r["kout"] for r in res.results], axis=0).view(np.uint32)
    lane = (15 - (kbits & np.uint32(15))).astype(np.intp)       # [ROWS, OUT_DIM]
    val = (kbits & np.uint32(0xFFFFFFF0)).view(np.float32)      # [ROWS, OUT_DIM]
    out = np.zeros((ROWS, OUT_DIM, DPC), np.float32)
    np.put_along_axis(out, lane[:, :, None], val[:, :, None], axis=2)
    return out.reshape(ROWS, N)
